# revision 23
# baseline (speedup 1.0000x reference)
"""Correlation1D Trainium2 Bass kernel.

out[b, d, h, w] = (1/C) * sum_c in1[b, c, h, w] * in2pad[b, c, h, w + d]
  B=8, C=256, H=96, W=192, PAD=40, D=81 displacement channels.

Strategy (data-parallel over batch, 1 sample per NeuronCore):
  Host pre-scales both inputs by 1/16 (exact power of two; folds the 1/C
  mean) and casts to fp16, halving HBM read traffic.  Per h row and
  96-wide w-chunk, PE matmuls (k=c, two 128-partition halves) build the
  valid 136 columns of the Gram band
      G[w, j] = sum_c in1[c, w] * in2[c, j]
  in PSUM (pad columns are zeroed once at startup and never rewritten).
  ACT/DVE copy PSUM -> SBUF band tiles (fp16).  The 81 output diagonals
  O[d, w] = band[w, w + d] are then pulled out by a DMA whose source
  access pattern has a fused partition+byte stride (+1 partition, +1
  element per step) -- the DMA reads each partition's 81-element diagonal
  run directly and writes DRAM in a [W, H, D] layout.  No GPSIMD scatter,
  no PE transposes.  The host reorders [W, H, D] -> [D, H, W] (a pure
  permutation) and upcasts to fp32.
"""

import os

import numpy as np

import concourse.bass as bass
import concourse.tile as tile
from concourse import bacc, mybir
from concourse.bass_utils import run_bass_kernel_spmd

# Problem constants (hardcoded per harness contract)
B = 8
C = 256
H = int(os.environ.get("CORR_H", "96"))
W = 192
PAD = 40
D = 2 * PAD + 1  # 81
DE = D  # D slot stride in the DRAM [W, H, DE] layout (contiguous)
CH = 2  # c split into CH partition-halves of 128
CP = C // CH  # 128
CHUNK = 96  # w-chunk (Gram output partition dim)
NCK = W // CHUNK  # 2
BANDW = CHUNK + D - 1  # 176 band columns per chunk
VALID = 136  # valid (non-pad) band columns per chunk
PAD_OFF = (40, 0)  # t-offset of valid region per chunk
JLO = (0, 56)  # first in2 column per chunk
GSTR = 256  # per-hl stride (elems) inside a PSUM g tile (bank-aligned)

# Tunables
SB = int(os.environ.get("CORR_SB", "16"))  # h rows per superblock
NSB = H // SB
NQ = SB // 4  # PSUM rounds per superblock
HH = 4  # hl rows per PSUM g tile (one round)
IN_BUFS = int(os.environ.get("CORR_IN_BUFS", "2"))
BAND_BUFS = int(os.environ.get("CORR_BAND_BUFS", "2"))

f16 = mybir.dt.float16
f32 = mybir.dt.float32


def _diag_src_ap(band_full, ck):
    """Source AP reading band[w, hl, ck, w + d] for hl in [0,SB), d in [0,D).

    band tile is [96, SB, NCK, BANDW] fp16.  The partition dim fuses a
    +1-partition, +1-element stride (flat stride SB*NCK*BANDW + 1), so the
    DMA walks each partition's diagonal 81-element run directly.
    """
    ap = band_full[:, :, ck, 0:D].copy()  # [[row,96],[NCK*BANDW,SB],[1,81]]
    row = SB * NCK * BANDW
    ap.ap[0] = [row + 1, CHUNK]
    return ap


def _build(reps=1):
    nc = bacc.Bacc("TRN2")

    in1 = nc.dram_tensor("input1", [C, H, W], f16, kind="ExternalInput")
    in2 = nc.dram_tensor("input2", [C, H, W], f16, kind="ExternalInput")
    # [W, H, DE] fp16; host permutes to [D, H, W] and upcasts
    out = nc.dram_tensor("out", [W, H, DE], f16, kind="ExternalOutput")

    # [c, h, w] -> [p, a, h*w] so each input load is one 3-dim DMA
    in1_r = in1.ap().rearrange("(a p) h w -> p a (h w)", p=CP)
    in2_r = in2.ap().rearrange("(a p) h w -> p a (h w)", p=CP)
    out_ap = out.ap()

    with tile.TileContext(nc) as tc:
        with (
            tc.tile_pool(name="loads", bufs=IN_BUFS) as loads,
            tc.tile_pool(name="bands", bufs=BAND_BUFS) as bands,
            tc.tile_pool(name="psg", bufs=1, space="PSUM") as psg,
        ):
            # Four persistent PSUM g tiles (one per (half, ck)), zeroed once.
            # Matmuls only ever write the valid 136-column regions; the pad
            # columns stay zero forever, so the evacuated band carries correct
            # zeros for the out-of-range displacements.
            g_tiles = {}
            for half in range(2):
                for ck in range(NCK):
                    g = psg.tile(
                        [CHUNK, HH, GSTR], f32, name=f"g_{half}_{ck}"
                    )
                    nc.vector.memset(g[:], 0.0)
                    g_tiles[(half, ck)] = g

            for _rep in range(reps):
                for sb in range(NSB):
                    h0 = sb * SB

                    # Both loads on the sync HWDGE ring: nothing else queues
                    # there, so prefetch is never FIFO-blocked behind a
                    # dependent DMA's wait.
                    in1_t = loads.tile([CP, CH, SB * W], f16)
                    nc.sync.dma_start(
                        out=in1_t[:], in_=in1_r[:, :, h0 * W : (h0 + SB) * W]
                    )
                    in2_t = loads.tile([CP, CH, SB * W], f16)
                    nc.sync.dma_start(
                        out=in2_t[:], in_=in2_r[:, :, h0 * W : (h0 + SB) * W]
                    )

                    band = bands.tile([CHUNK, SB, NCK, BANDW], f16)

                    for q in range(NQ):
                        for ck in range(NCK):
                            g = g_tiles[(q % 2, ck)]
                            po = PAD_OFF[ck]
                            for hh in range(HH):
                                hl = q * HH + hh
                                for a in range(CH):
                                    nc.tensor.matmul(
                                        g[:, hh, po : po + VALID],
                                        in1_t[
                                            :, a,
                                            hl * W + ck * CHUNK
                                            : hl * W + (ck + 1) * CHUNK,
                                        ],
                                        in2_t[
                                            :, a,
                                            hl * W + JLO[ck]
                                            : hl * W + JLO[ck] + VALID,
                                        ],
                                        start=(a == 0),
                                        stop=(a == CH - 1),
                                    )
                            # evacuate PSUM -> band (fp32 -> fp16); split the
                            # (q, ck) copies across ACT and DVE
                            src = g[:, :, 0:BANDW]
                            dst = band[:, q * HH : (q + 1) * HH, ck, :]
                            if ck == 0:
                                nc.scalar.copy(out=dst, in_=src)
                            else:
                                nc.vector.tensor_copy(out=dst, in_=src)

                    # Diagonal DMAs straight to DRAM: band[w, hl, ck, w+d] ->
                    # out[96ck+w, h0+hl, d].  Source runs are 162 B, but for a
                    # fixed partition the 16 hl runs land address-contiguously
                    # in DRAM (2592 B per partition), so the HBM writes merge.
                    # Issued on the scalar HWDGE ring, *behind* the ACT evacs
                    # that produce the band -- the wait never blocks loads.
                    for ck in range(NCK):
                        nc.scalar.dma_start(
                            out=out_ap[
                                ck * CHUNK : (ck + 1) * CHUNK,
                                h0 : h0 + SB,
                                0:D,
                            ],
                            in_=_diag_src_ap(band, ck),
                        )

    nc.compile()
    return nc


_NC_CACHE = {}


def _get_nc(reps=1):
    if reps not in _NC_CACHE:
        _NC_CACHE[reps] = _build(reps)
    return _NC_CACHE[reps]


def make_diag_idx():  # kept for test.py compat; no longer a kernel input
    return None


def run(input1, input2, trace=False, reps=1, **spmd_kwargs):
    """Run on 8 NeuronCores; returns (out [B,D,H,W] fp32, BassKernelResults)."""
    nc = _get_nc(reps)

    input1 = np.asarray(input1)
    input2 = np.asarray(input2)
    assert input1.shape == (B, C, H, W) and input2.shape == (B, C, H, W)
    # 1/16 per input folds the 1/C=1/256 mean; exact power-of-two scales
    in1h = np.ascontiguousarray((input1 * np.float32(1 / 16)).astype(np.float16))
    in2h = np.ascontiguousarray((input2 * np.float32(1 / 16)).astype(np.float16))

    in_maps = [{"input1": in1h[b], "input2": in2h[b]} for b in range(B)]
    res = run_bass_kernel_spmd(
        nc, in_maps, core_ids=list(range(B)), trace=trace, **spmd_kwargs
    )
    # [W, H, DE] fp16 -> [D, H, W] fp32
    out = np.stack(
        [
            np.ascontiguousarray(
                np.transpose(res.results[b]["out"][:, :, 0:D], (2, 1, 0))
            ).astype(np.float32)
            for b in range(B)
        ],
        axis=0,
    )
    return out, res


def kernel(input1, input2):
    out, _ = run(input1, input2)
    return out


# revision 26
# speedup vs baseline: 1.7739x; 1.7739x over previous
"""Correlation1D Trainium2 Bass kernel.

out[b, d, h, w] = (1/C) * sum_c in1[b, c, h, w] * in2pad[b, c, h, w + d]
  B=8, C=256, H=96, W=192, PAD=40, D=81 displacement channels.

Strategy (data-parallel over batch, 1 sample per NeuronCore):
  Host pre-scales both inputs by 1/16 (exact power of two; folds the 1/C
  mean) and casts to fp16, halving HBM read traffic.  Per h row and
  96-wide w-chunk, PE matmuls (k=c, two 128-partition halves) build the
  valid 136 columns of the Gram band
      G[w, j] = sum_c in1[c, w] * in2[c, j]
  in PSUM (pad columns are zeroed once at startup and never rewritten).
  ACT/DVE copy PSUM -> SBUF band tiles (fp16).  The 81 output diagonals
  O[d, w] = band[w, w + d] are then pulled out by a DMA whose source
  access pattern has a fused partition+byte stride (+1 partition, +1
  element per step) -- the DMA reads each partition's 81-element diagonal
  run directly and writes DRAM in a [W, H, D] layout.  No GPSIMD scatter,
  no PE transposes.  The host reorders [W, H, D] -> [D, H, W] (a pure
  permutation) and upcasts to fp32.
"""

import os

import numpy as np

import concourse.bass as bass
import concourse.tile as tile
from concourse import bacc, mybir
from concourse.bass_utils import run_bass_kernel_spmd

# Problem constants (hardcoded per harness contract)
B = 8
C = 256
H = int(os.environ.get("CORR_H", "96"))
W = 192
PAD = 40
D = 2 * PAD + 1  # 81
DE = D  # D slot stride in the DRAM [W, H, DE] layout (contiguous)
CH = 2  # c split into CH partition-halves of 128
CP = C // CH  # 128
CHUNK = 96  # w-chunk (Gram output partition dim)
NCK = W // CHUNK  # 2
BANDW = CHUNK + D - 1  # 176 band columns per chunk
VALID = 136  # valid (non-pad) band columns per chunk
PAD_OFF = (40, 0)  # t-offset of valid region per chunk
JLO = (0, 56)  # first in2 column per chunk
GSTR = 256  # per-hl stride (elems) inside a PSUM g tile (bank-aligned)

# Tunables
SB = int(os.environ.get("CORR_SB", "16"))  # h rows per superblock
NSB = H // SB
NQ = SB // 4  # PSUM rounds per superblock
HH = 4  # hl rows per PSUM g tile (one round)
IN_BUFS = int(os.environ.get("CORR_IN_BUFS", "2"))
BAND_BUFS = int(os.environ.get("CORR_BAND_BUFS", "2"))

f16 = mybir.dt.float16
f32 = mybir.dt.float32


def _diag_src_ap(band_full, ck):
    """Source AP reading band[w, hl, ck, w + d] for hl in [0,SB), d in [0,D).

    band tile is [96, SB, NCK, BANDW] fp16.  The partition dim fuses a
    +1-partition, +1-element stride (flat stride SB*NCK*BANDW + 1), so the
    DMA walks each partition's diagonal 81-element run directly.
    """
    ap = band_full[:, :, ck, 0:D].copy()  # [[row,96],[NCK*BANDW,SB],[1,81]]
    row = SB * NCK * BANDW
    ap.ap[0] = [row + 1, CHUNK]
    return ap


def _build(reps=1):
    nc = bacc.Bacc("TRN2")

    in1 = nc.dram_tensor("input1", [C, H, W], f16, kind="ExternalInput")
    in2 = nc.dram_tensor("input2", [C, H, W], f16, kind="ExternalInput")
    # [W, H, DE] fp16; host permutes to [D, H, W] and upcasts
    out = nc.dram_tensor("out", [W, H, DE], f16, kind="ExternalOutput")

    # [c, h, w] -> [p, a, h*w] so each input load is one 3-dim DMA
    in1_r = in1.ap().rearrange("(a p) h w -> p a (h w)", p=CP)
    in2_r = in2.ap().rearrange("(a p) h w -> p a (h w)", p=CP)
    out_ap = out.ap()

    with tile.TileContext(nc) as tc:
        with (
            tc.tile_pool(name="loads", bufs=IN_BUFS) as loads,
            tc.tile_pool(name="bands", bufs=BAND_BUFS) as bands,
            tc.tile_pool(name="gats", bufs=BAND_BUFS) as gats,
            tc.tile_pool(name="psg", bufs=1, space="PSUM") as psg,
        ):
            # Four persistent PSUM g tiles (one per (half, ck)), zeroed once.
            # Matmuls only ever write the valid 136-column regions; the pad
            # columns stay zero forever, so the evacuated band carries correct
            # zeros for the out-of-range displacements.
            g_tiles = {}
            for half in range(2):
                for ck in range(NCK):
                    g = psg.tile(
                        [CHUNK, HH, GSTR], f32, name=f"g_{half}_{ck}"
                    )
                    nc.vector.memset(g[:], 0.0)
                    g_tiles[(half, ck)] = g

            for _rep in range(reps):
                for sb in range(NSB):
                    h0 = sb * SB

                    # One load per HWDGE ring; nothing else queues on these
                    # rings, so prefetch is never FIFO-blocked behind a
                    # dependent DMA's wait.
                    in1_t = loads.tile([CP, CH, SB * W], f16)
                    nc.sync.dma_start(
                        out=in1_t[:], in_=in1_r[:, :, h0 * W : (h0 + SB) * W]
                    )
                    in2_t = loads.tile([CP, CH, SB * W], f16)
                    nc.scalar.dma_start(
                        out=in2_t[:], in_=in2_r[:, :, h0 * W : (h0 + SB) * W]
                    )

                    band = bands.tile([CHUNK, SB, NCK, BANDW], f16)

                    for q in range(NQ):
                        for ck in range(NCK):
                            g = g_tiles[(q % 2, ck)]
                            po = PAD_OFF[ck]
                            for hh in range(HH):
                                hl = q * HH + hh
                                for a in range(CH):
                                    nc.tensor.matmul(
                                        g[:, hh, po : po + VALID],
                                        in1_t[
                                            :, a,
                                            hl * W + ck * CHUNK
                                            : hl * W + (ck + 1) * CHUNK,
                                        ],
                                        in2_t[
                                            :, a,
                                            hl * W + JLO[ck]
                                            : hl * W + JLO[ck] + VALID,
                                        ],
                                        start=(a == 0),
                                        stop=(a == CH - 1),
                                    )
                            # evacuate PSUM -> band (fp32 -> fp16); split the
                            # (q, ck) copies across ACT and DVE
                            src = g[:, :, 0:BANDW]
                            dst = band[:, q * HH : (q + 1) * HH, ck, :]
                            if ck == 0:
                                nc.scalar.copy(out=dst, in_=src)
                            else:
                                nc.vector.tensor_copy(out=dst, in_=src)

                    # Diagonal extraction in two DMA stages, all on the SWDGE
                    # (GPSIMD) queue so the load rings stay clean and stage 2's
                    # wait sits behind its own producers in FIFO order.
                    # Stage 1 walks the band diagonals band[w, hl, ck, w+d]
                    # SBUF->SBUF into a packed gat tile -- its 162 B runs
                    # aggregate into ~4 KB packets on the fabric side (the
                    # non-aggregating small-packet penalty is HBM-only; direct
                    # diag->HBM measured 1.5 GB/s-busy per engine).  Stage 2
                    # writes DRAM out[96ck+w, h0+hl, d] as one DMA with
                    # contiguous 2592 B runs per (w, ck).
                    gat = gats.tile(
                        [CHUNK, NCK, SB, D], f16,
                        name=f"gat_{_rep}_{sb}", tag="gat",
                    )
                    for ck in range(NCK):
                        nc.gpsimd.dma_start(
                            out=gat[:, ck, :, :], in_=_diag_src_ap(band, ck)
                        )
                    out_dst = out_ap.rearrange(
                        "(k w) h d -> w k h d", k=NCK
                    )[:, :, h0 : h0 + SB, 0:D]
                    nc.gpsimd.dma_start(out=out_dst, in_=gat[:])

    nc.compile()
    return nc


_NC_CACHE = {}


def _get_nc(reps=1):
    if reps not in _NC_CACHE:
        _NC_CACHE[reps] = _build(reps)
    return _NC_CACHE[reps]


def make_diag_idx():  # kept for test.py compat; no longer a kernel input
    return None


def run(input1, input2, trace=False, reps=1, **spmd_kwargs):
    """Run on 8 NeuronCores; returns (out [B,D,H,W] fp32, BassKernelResults)."""
    nc = _get_nc(reps)

    input1 = np.asarray(input1)
    input2 = np.asarray(input2)
    assert input1.shape == (B, C, H, W) and input2.shape == (B, C, H, W)
    # 1/16 per input folds the 1/C=1/256 mean; exact power-of-two scales
    in1h = np.ascontiguousarray((input1 * np.float32(1 / 16)).astype(np.float16))
    in2h = np.ascontiguousarray((input2 * np.float32(1 / 16)).astype(np.float16))

    in_maps = [{"input1": in1h[b], "input2": in2h[b]} for b in range(B)]
    res = run_bass_kernel_spmd(
        nc, in_maps, core_ids=list(range(B)), trace=trace, **spmd_kwargs
    )
    # [W, H, DE] fp16 -> [D, H, W] fp32
    out = np.stack(
        [
            np.ascontiguousarray(
                np.transpose(res.results[b]["out"][:, :, 0:D], (2, 1, 0))
            ).astype(np.float32)
            for b in range(B)
        ],
        axis=0,
    )
    return out, res


def kernel(input1, input2):
    out, _ = run(input1, input2)
    return out
